# revision 1
# baseline (speedup 1.0000x reference)
"""Trainium2 Bass kernel for nn_DecoderLayer (Transformer-XL style decoder layer).

Sharding (8 cores = 2 batch groups x 4-way tensor parallel):
  core c: b = c // 4, g = c % 4
  - Attention: head-parallel. Each core computes its 4 heads (of 16) for its
    batch: Q^T/K^T via column-parallel Wq/Wkv; scores S^T[j, t] on PE; exp on
    ACT; P^T against [V|1] accumulates attn_vec^T plus the softmax denominator
    in one PSUM group; row-parallel Wo gives a partial attn_out.
  - ReduceScatter over each 4-core group sums the Wo partials and scatters
    t-rows: core g receives rows [512g, 512g+512).
  - FF is sequence-parallel on the core's own 512 rows with full W1/W2.
  - Causal structure: score blocks with j > t+M are never computed; boundary
    blocks get an additive -1e9 mask built on the host from the actual
    attn_mask input (arbitrary masks fall back to more mask blocks).
All matmuls in bf16 with fp32 PSUM accumulation; softmax/LN in fp32.
"""

import sys

sys.path.insert(0, "/opt/trn_rl_repo")

from contextlib import ExitStack

import numpy as np
import ml_dtypes

import concourse.bass as bass
import concourse.bacc as bacc
import concourse.mybir as mybir
import concourse.tile as tile
from concourse.bass_utils import run_bass_kernel_spmd
from concourse.masks import make_identity

T, M, B, D, H, DH, DI = 2048, 1024, 2, 1024, 16, 64, 4096
TM = T + M
NCORES = 8
G = 4                # tensor-parallel group size
HL = H // G          # 4 local heads
HDH_L = HL * DH      # 256 local q/k/v features
TQ = T // G          # 512 t-rows per core after ReduceScatter
NB_J = TM // 128     # 24 key blocks
NEG = -1.0e9
SCALE = 1.0 / float(DH) ** 0.5

BF16 = mybir.dt.bfloat16
F32 = mybir.dt.float32
NPBF16 = ml_dtypes.bfloat16

_prog_cache = {}


def _bf(x):
    return np.ascontiguousarray(np.asarray(x, dtype=np.float32).astype(NPBF16))


def _f32(x):
    return np.ascontiguousarray(np.asarray(x, dtype=np.float32))


def build_program(fvt, mask_list, trace=False):
    """fvt[bj] = first visible t-block (0..16; 16 = column fully masked).
    mask_list = tuple of (bj, tb) pairs needing an additive mask tile."""
    fvt = list(fvt)
    n_mask = max(len(mask_list), 1)
    AF = mybir.ActivationFunctionType
    ALU = mybir.AluOpType

    nc = bacc.Bacc(None, target_bir_lowering=False, num_devices=NCORES)

    ct_d = nc.dram_tensor("ct", [8, 128, TM], BF16, kind="ExternalInput")
    hres_d = nc.dram_tensor("hres", [4, 128, D], F32, kind="ExternalInput")
    wq_d = nc.dram_tensor("wq", [8, 128, HDH_L], BF16, kind="ExternalInput")
    wkv_d = nc.dram_tensor("wkv", [8, 128, 2 * HDH_L], BF16, kind="ExternalInput")
    wo_d = nc.dram_tensor("wo", [2, 128, D], BF16, kind="ExternalInput")
    w1_d = nc.dram_tensor("w1", [32, 128, 8, 128], BF16, kind="ExternalInput")
    w2_d = nc.dram_tensor("w2", [32, 128, D], BF16, kind="ExternalInput")
    mask_d = nc.dram_tensor("maskt", [n_mask, 128, 128], F32, kind="ExternalInput")
    b1_d = nc.dram_tensor("b1t", [128, 32], F32, kind="ExternalInput")
    b2_d = nc.dram_tensor("b2b", [128, D], F32, kind="ExternalInput")
    ln_d = nc.dram_tensor("lnp", [4, 128, D], F32, kind="ExternalInput")
    out_d = nc.dram_tensor("out", [4, 128, D], F32, kind="ExternalOutput")

    # last contributing bj per 512-wide accumulator piece (for stop= flags)
    last_bj = []
    for p in range(4):
        contrib = [bj for bj in range(NB_J) if fvt[bj] * 128 < (p + 1) * 512]
        last_bj.append(contrib[-1] if contrib else -1)

    mask_by_bj = {}
    for i, (bj, tb) in enumerate(mask_list):
        mask_by_bj.setdefault(bj, []).append((tb, i))

    with ExitStack() as ctx:
        tc = ctx.enter_context(tile.TileContext(nc))
        per = ctx.enter_context(tc.tile_pool(name="per", bufs=1))
        attn_cm = tc.tile_pool(name="attn", bufs=1)
        attn = attn_cm.__enter__()
        dram = ctx.enter_context(tc.tile_pool(name="dram", bufs=1, space="DRAM"))

        # ---- attention-lifetime SBUF tiles (pool closed after stage C)
        qT = [attn.tile([128, T], BF16, tag=f"qT{m}", name=f"qT{m}") for m in range(2)]
        kvT = [attn.tile([128, TM], BF16, tag=f"kvT{m}", name=f"kvT{m}") for m in range(4)]
        v_s = [attn.tile([128, HL, DH + 1], BF16, tag=f"v{jb}", name=f"v{jb}") for jb in range(NB_J)]
        avT = [attn.tile([128, T], BF16, tag=f"avT{m}", name=f"avT{m}") for m in range(2)]
        wo_s = [attn.tile([128, D], BF16, tag=f"wo{m}", name=f"wo{m}") for m in range(2)]
        mk_s = [attn.tile([128, 128], F32, tag=f"mk{i}", name=f"mk{i}") for i in range(len(mask_list))]
        b1_s = per.tile([128, 32], F32, tag="b1", name="b1")
        b2_s = per.tile([128, D], F32, tag="b2", name="b2")
        ln_s = [per.tile([128, D], F32, tag=f"ln{i}", name=f"ln{i}") for i in range(4)]
        hres_s = [per.tile([128, D], F32, tag=f"hres{i}", name=f"hres{i}") for i in range(4)]
        ones_s = attn.tile([1, 64], BF16, tag="ones", name="ones")
        eps_s = per.tile([128, 1], F32, tag="eps", name="eps")
        z65_s = attn.tile([128, 65], BF16, tag="z65", name="z65")
        zrhs_s = attn.tile([128, 512], BF16, tag="zrhs", name="zrhs")
        rec_s = attn.tile([1, T], F32, tag="rec", name="rec")
        recb_s = attn.tile([1, T], BF16, tag="recb", name="recb")
        ident = per.tile([128, 128], BF16, tag="ident", name="ident")

        rs_in = dram.tile([16, 128, D], F32, tag="rsin", name="rsin")
        rs_out = dram.tile([4, 128, D], F32, tag="rsout", name="rsout")

        nc.vector.memset(ones_s[:], 1.0)
        nc.vector.memset(eps_s[:], 1e-5)
        nc.vector.memset(z65_s[:], 0.0)
        nc.vector.memset(zrhs_s[:], 0.0)
        make_identity(nc, ident[:])

        # ================= Stage A: projections =================
        with tc.tile_pool(name="ctp", bufs=1) as ctp, \
             tc.tile_pool(name="wp", bufs=1) as wp, \
             tc.tile_pool(name="psA", bufs=2, space="PSUM") as psA:
            ct_s = [ctp.tile([128, TM], BF16, tag=f"ct{kd}", name=f"ct{kd}") for kd in range(8)]
            wq_s = [wp.tile([128, HDH_L], BF16, tag=f"wq{kd}", name=f"wq{kd}") for kd in range(8)]
            wkv_s = [wp.tile([128, 2 * HDH_L], BF16, tag=f"wkv{kd}", name=f"wkv{kd}")
                     for kd in range(8)]
            for kd in range(8):
                nc.sync.dma_start(out=ct_s[kd][:], in_=ct_d[kd])
                nc.sync.dma_start(out=wq_s[kd][:], in_=wq_d[kd])
                nc.sync.dma_start(out=wkv_s[kd][:], in_=wkv_d[kd])

            # parameter DMAs emitted after the critical-path inputs
            for m in range(2):
                nc.sync.dma_start(out=wo_s[m][:], in_=wo_d[m])
            for i in range(len(mask_list)):
                nc.sync.dma_start(out=mk_s[i][:], in_=mask_d[i])
            nc.sync.dma_start(out=b1_s[:], in_=b1_d[:])
            nc.sync.dma_start(out=b2_s[:], in_=b2_d[:])
            for i in range(4):
                nc.sync.dma_start(out=ln_s[i][:], in_=ln_d[i])
                nc.sync.dma_start(out=hres_s[i][:], in_=hres_d[i])

            # qT[m][:, n*512:+512] = sum_kd wq[kd][:, m-cols].T @ hT-part
            for m in range(2):
                for n in range(4):
                    pq = psA.tile([128, 512], F32, tag="pa", name="pa")
                    for kd in range(8):
                        nc.tensor.matmul(
                            pq[:],
                            wq_s[kd][:, m * 128:(m + 1) * 128],
                            ct_s[kd][:, M + n * 512: M + (n + 1) * 512],
                            start=(kd == 0), stop=(kd == 7),
                        )
                    nc.vector.tensor_copy(qT[m][:, n * 512:(n + 1) * 512], pq[:])
            for m in range(4):
                for n in range(6):
                    pkv = psA.tile([128, 512], F32, tag="pa", name="pa")
                    for kd in range(8):
                        nc.tensor.matmul(
                            pkv[:],
                            wkv_s[kd][:, m * 128:(m + 1) * 128],
                            ct_s[kd][:, n * 512:(n + 1) * 512],
                            start=(kd == 0), stop=(kd == 7),
                        )
                    nc.vector.tensor_copy(kvT[m][:, n * 512:(n + 1) * 512], pkv[:])

            # V natural layout via PE transpose of kvT rows 256..511
            for jb in range(NB_J):
                nc.vector.memset(v_s[jb][:, :, DH:DH + 1], 1.0)
                for vb in range(2):
                    ptr = psA.tile([128, 128], BF16, tag="ptr", name="ptr")
                    nc.tensor.transpose(
                        ptr[:], kvT[2 + vb][:, jb * 128:(jb + 1) * 128], ident[:]
                    )
                    for c_ in range(2):
                        h_loc = 2 * vb + c_
                        nc.vector.tensor_copy(
                            v_s[jb][:, h_loc, 0:DH], ptr[:, c_ * 64:(c_ + 1) * 64]
                        )

        # ========= Stage B+C: attention, Wo, chunked ReduceScatter =========
        # Split over t-halves: Wo + RS for half 0 overlap attention of half 1.
        with tc.tile_pool(name="psBs", bufs=2, space="PSUM") as psBs, \
             tc.tile_pool(name="psBa", bufs=1, space="PSUM") as psBa, \
             tc.tile_pool(name="psC", bufs=2, space="PSUM") as psC, \
             tc.tile_pool(name="ptp", bufs=3) as ptp, \
             tc.tile_pool(name="bcp", bufs=2) as bcp, \
             tc.tile_pool(name="aop", bufs=3) as aop:
            for half in range(2):
                th0, th1 = half * 1024, (half + 1) * 1024
                # last contributing bj per absolute 512-piece in this half
                lastb = []
                for p in range(2):
                    pe_end = th0 + (p + 1) * 512
                    contrib = [bj for bj in range(NB_J)
                               if fvt[bj] < 16 and fvt[bj] * 128 < pe_end]
                    lastb.append(contrib[-1] if contrib else -1)
                for h in range(HL):
                    hp, ho = h // 2, (h % 2) * 64
                    acc = psBa.tile([65, 1024], F32, tag="acc", name="acc")
                    for p in range(2):
                        nc.tensor.matmul(
                            acc[:, p * 512:(p + 1) * 512], z65_s[:], zrhs_s[:],
                            start=True, stop=(lastb[p] < 0),
                        )

                    def emit_scores(bj):
                        qs, qe = max(fvt[bj] * 128, th0), th1
                        sp = psBs.tile([128, 1024], F32, tag="sp", name="sp")
                        ss = qs
                        while ss < qe:
                            se = min(qe, ss + 512)
                            nc.tensor.matmul(
                                sp[:, ss - qs: se - qs],
                                kvT[hp][ho:ho + 64, bj * 128:(bj + 1) * 128],
                                qT[hp][ho:ho + 64, ss:se],
                                start=True, stop=True,
                            )
                            ss = se
                        for tb, mi in mask_by_bj.get(bj, []):
                            c0 = tb * 128
                            if qs <= c0 < qe:
                                nc.vector.tensor_add(
                                    sp[:, c0 - qs: c0 - qs + 128],
                                    sp[:, c0 - qs: c0 - qs + 128],
                                    mk_s[mi][:],
                                )
                        return bj, qs, qe, sp

                    def emit_exp_pv(job):
                        bj, qs, qe, sp = job
                        pt = ptp.tile([128, 1024], BF16, tag="pt", name="pt")
                        nc.scalar.activation(
                            pt[:, 0: qe - qs], sp[:, 0: qe - qs], AF.Exp,
                            bias=0.0, scale=SCALE,
                        )
                        ss = qs
                        while ss < qe:
                            se = min(qe, (ss // 512 + 1) * 512)
                            p = (ss - th0) // 512
                            nc.tensor.matmul(
                                acc[:, ss - th0: se - th0],
                                v_s[bj][:, h, :],
                                pt[:, ss - qs: se - qs],
                                start=False, stop=(bj == lastb[p]),
                            )
                            ss = se

                    # software pipeline: emit S(bj+1) before exp/PV(bj) so the
                    # in-order PE stream never stalls on ACT's exp latency
                    pend = None
                    for bj in range(NB_J):
                        if fvt[bj] >= 16 or fvt[bj] * 128 >= th1:
                            continue
                        cur = emit_scores(bj)
                        if pend is not None:
                            emit_exp_pv(pend)
                        pend = cur
                    if pend is not None:
                        emit_exp_pv(pend)
                    # normalize: attn_vec^T * (1/denom)
                    nc.vector.reciprocal(rec_s[:, th0:th1], acc[64:65, :])
                    nc.vector.tensor_copy(recb_s[:, th0:th1], rec_s[:, th0:th1])
                    for p in range(2):
                        a0 = th0 + p * 512
                        bc_ps = psBs.tile([64, 512], F32, tag="sp", name="sp")
                        nc.tensor.matmul(
                            bc_ps[:], ones_s[:], recb_s[:, a0:a0 + 512],
                            start=True, stop=True,
                        )
                        bc = bcp.tile([64, 512], F32, tag="bc", name="bc")
                        nc.vector.tensor_copy(bc[:], bc_ps[:])
                        nc.vector.tensor_mul(
                            avT[hp][ho:ho + 64, a0:a0 + 512],
                            acc[0:64, p * 512:(p + 1) * 512],
                            bc[:],
                        )
                # Wo partials for this half + RS chunks
                for tcb in range(8 * half, 8 * half + 8):
                    ao = aop.tile([128, D], F32, tag="ao", name="ao")
                    for nn in range(2):
                        po = psC.tile([128, 512], F32, tag="po", name="po")
                        for hp in range(2):
                            nc.tensor.matmul(
                                po[:],
                                avT[hp][:, tcb * 128:(tcb + 1) * 128],
                                wo_s[hp][:, nn * 512:(nn + 1) * 512],
                                start=(hp == 0), stop=(hp == 1),
                            )
                        nc.vector.tensor_copy(ao[:, nn * 512:(nn + 1) * 512], po[:])
                    nc.sync.dma_start(out=rs_in[tcb], in_=ao[:])
                    if tcb % 4 == 3:
                        q = tcb // 4
                        nc.gpsimd.collective_compute(
                            "ReduceScatter", mybir.AluOpType.add,
                            replica_groups=[[0, 1, 2, 3], [4, 5, 6, 7]],
                            ins=[rs_in[4 * q: 4 * (q + 1)].opt()],
                            outs=[rs_out[q: q + 1].opt()],
                        )

        attn_cm.__exit__(None, None, None)

        # ============ Stage D: LN1, FF, LN2, out ============
        def layernorm(x_out, x_in, g_sb, b_sb, sp_pool):
            st = sp_pool.tile([128, 2, 6], F32, tag="bnst", name="bnst")
            for s in range(2):
                nc.vector.bn_stats(out=st[:, s, :],
                                   in_=x_in[:, s * 512:(s + 1) * 512])
            mv = sp_pool.tile([128, 2], F32, tag="bnmv", name="bnmv")
            nc.vector.bn_aggr(out=mv[:], in_=st[:])
            nc.scalar.activation(
                out=mv[:, 1:2], in_=mv[:, 1:2], func=AF.Sqrt,
                bias=eps_s[:, 0:1], scale=1.0,
            )
            nc.vector.reciprocal(out=mv[:, 1:2], in_=mv[:, 1:2])
            nc.vector.tensor_scalar(
                out=x_out, in0=x_in, scalar1=mv[:, 0:1], scalar2=mv[:, 1:2],
                op0=ALU.subtract, op1=ALU.mult,
            )
            nc.vector.tensor_mul(x_out, x_out, g_sb)
            nc.vector.tensor_add(x_out, x_out, b_sb)

        with tc.tile_pool(name="sdp", bufs=1) as sdp, \
             tc.tile_pool(name="sd", bufs=3) as sd:
            xT = [sdp.tile([128, TQ], BF16, tag=f"xT{k}", name=f"xT{k}") for k in range(8)]
            rT = [sdp.tile([128, TQ], BF16, tag=f"rT{k}", name=f"rT{k}") for k in range(32)]
            x_s = [sdp.tile([128, D], F32, tag=f"x{k}", name=f"x{k}") for k in range(4)]

            pstr_cm = tc.tile_pool(name="pstr", bufs=2, space="PSUM")
            pstr = pstr_cm.__enter__()
            for k4 in range(4):
                asum = sd.tile([128, D], F32, tag="asum", name="asum")
                nc.sync.dma_start(out=asum[:], in_=rs_out[k4])
                xin = sd.tile([128, D], F32, tag="xin", name="xin")
                nc.vector.tensor_add(xin[:], asum[:], hres_s[k4][:])
                layernorm(x_s[k4][:], xin[:], ln_s[0][:], ln_s[1][:], sd)
                xbf = sd.tile([128, D], BF16, tag="xbf", name="xbf")
                nc.vector.tensor_copy(xbf[:], x_s[k4][:])
                for kd in range(8):
                    ptr = pstr.tile([128, 128], BF16, tag="tr", name="tr")
                    nc.tensor.transpose(
                        ptr[:], xbf[:, kd * 128:(kd + 1) * 128], ident[:]
                    )
                    nc.vector.tensor_copy(
                        xT[kd][:, k4 * 128:(k4 + 1) * 128], ptr[:]
                    )

            pstr_cm.__exit__(None, None, None)

            # FF1: rT[dic] = relu(W1[:, dic].T @ x^T + b1)
            with tc.tile_pool(name="w1p", bufs=3) as w1p, \
                 tc.tile_pool(name="psf1", bufs=3, space="PSUM") as psf1:
                for dic in range(32):
                    w1t = w1p.tile([128, 8, 128], BF16, tag="w1t", name="w1t")
                    nc.sync.dma_start(out=w1t[:], in_=w1_d[dic])
                    f1 = psf1.tile([128, TQ], F32, tag="f1", name="f1")
                    for kd in range(8):
                        nc.tensor.matmul(
                            f1[:], w1t[:, kd, :], xT[kd][:],
                            start=(kd == 0), stop=(kd == 7),
                        )
                    nc.scalar.activation(
                        out=rT[dic][:], in_=f1[:], func=AF.Relu,
                        bias=b1_s[:, dic:dic + 1], scale=1.0,
                    )

            # FF2 uses all 8 PSUM banks (4 accumulators of [128, 1024] f32)
            with tc.tile_pool(name="psF", bufs=1, space="PSUM") as psF, \
                 tc.tile_pool(name="w2p", bufs=3) as w2p, \
                 tc.tile_pool(name="se", bufs=2) as se:
                f2 = [psF.tile([128, D], F32, tag=f"f2_{k}", name=f"f2_{k}") for k in range(4)]
                for dic in range(32):
                    w2t = w2p.tile([128, D], BF16, tag="w2t", name="w2t")
                    nc.sync.dma_start(out=w2t[:], in_=w2_d[dic])
                    for k4 in range(4):
                        for nn in range(2):
                            nc.tensor.matmul(
                                f2[k4][:, nn * 512:(nn + 1) * 512],
                                rT[dic][:, k4 * 128:(k4 + 1) * 128],
                                w2t[:, nn * 512:(nn + 1) * 512],
                                start=(dic == 0), stop=(dic == 31),
                            )
                for k4 in range(4):
                    x2 = se.tile([128, D], F32, tag="x2", name="x2")
                    nc.vector.tensor_add(x2[:], f2[k4][:], x_s[k4][:])
                    nc.vector.tensor_add(x2[:], x2[:], b2_s[:])
                    o = se.tile([128, D], F32, tag="o", name="o")
                    layernorm(o[:], x2[:], ln_s[2][:], ln_s[3][:], se)
                    nc.sync.dma_start(out=out_d[k4], in_=o[:])

    nc.compile()
    return nc


def _analyze_mask(attn_mask):
    """Derive block structure from the actual mask input."""
    mb = attn_mask  # [T, TM, B] bool
    any_vis = ~mb.all(axis=2)  # visible in at least one batch
    nb_t = T // 128
    fvt = []
    for bj in range(NB_J):
        col = any_vis[:, bj * 128:(bj + 1) * 128]
        vis_tb = [tb for tb in range(nb_t)
                  if col[tb * 128:(tb + 1) * 128, :].any()]
        fvt.append(vis_tb[0] if vis_tb else 16)
    m_any = attn_mask.any(axis=2)
    need = []
    for bj in range(NB_J):
        if fvt[bj] >= 16:
            continue
        for tb in range(fvt[bj], nb_t):
            if m_any[tb * 128:(tb + 1) * 128, bj * 128:(bj + 1) * 128].any():
                need.append((bj, tb))
    return tuple(fvt), tuple(need)


def _prep_inputs(dec_inp, attn_mask, mems, Wq, Wkv, Wo, ln1_g, ln1_b, W1, b1,
                 W2, b2, ln2_g, ln2_b, mask_list):
    c_full = np.concatenate([_f32(mems), _f32(dec_inp)], axis=0)  # [TM, B, D]
    # w1_r[dic][p, kd, :] = W1[kd*128+p, dic*128:+128]
    w1_r = _bf(np.asarray(W1, np.float32).reshape(8, 128, 32, 128)
               .transpose(2, 1, 0, 3))
    w2_r = _bf(np.asarray(W2, np.float32).reshape(32, 128, D))
    b1t = _f32(np.asarray(b1).reshape(32, 128).T)
    b2b = _f32(np.broadcast_to(np.asarray(b2)[None, :], (128, D)))
    lnp = _f32(np.stack([np.broadcast_to(np.asarray(v, np.float32)[None, :],
                                         (128, D))
                         for v in (ln1_g, ln1_b, ln2_g, ln2_b)]))
    dec32 = _f32(dec_inp)
    in_maps = []
    for core in range(NCORES):
        b, g = core // G, core % G
        ctb = _bf(c_full[:, b, :].T.reshape(8, 128, TM))
        rows = np.concatenate(
            [np.arange(512 * q + 128 * g, 512 * q + 128 * g + 128)
             for q in range(4)])
        hres = _f32(dec32[rows, b, :]).reshape(4, 128, D)
        wq_c = _bf(np.asarray(Wq)[:, g * HDH_L:(g + 1) * HDH_L]
                   .reshape(8, 128, HDH_L))
        wkv_c = _bf(np.concatenate(
            [np.asarray(Wkv)[:, g * HDH_L:(g + 1) * HDH_L],
             np.asarray(Wkv)[:, H * DH + g * HDH_L: H * DH + (g + 1) * HDH_L]],
            axis=1).reshape(8, 128, 2 * HDH_L))
        wo_c = _bf(np.asarray(Wo)[g * HDH_L:(g + 1) * HDH_L, :]
                   .reshape(2, 128, D))
        n_mask = max(len(mask_list), 1)
        mk = np.zeros((n_mask, 128, 128), np.float32)
        for i, (bj, tb) in enumerate(mask_list):
            blk = attn_mask[tb * 128:(tb + 1) * 128,
                            bj * 128:(bj + 1) * 128, b]
            mk[i] = np.where(blk.T, NEG, 0.0).astype(np.float32)
        in_maps.append({
            "ct": ctb, "hres": hres, "wq": wq_c, "wkv": wkv_c, "wo": wo_c,
            "w1": w1_r, "w2": w2_r, "maskt": mk, "b1t": b1t, "b2b": b2b,
            "lnp": lnp,
        })
    return in_maps


def kernel(dec_inp, attn_mask, mems, Wq, Wkv, Wo, ln1_g, ln1_b, W1, b1, W2, b2,
           ln2_g, ln2_b, _trace=False, _trace_kwargs=None):
    attn_mask = np.asarray(attn_mask).astype(bool)
    fvt, mask_list = _analyze_mask(attn_mask)
    key = (fvt, mask_list)
    if key not in _prog_cache:
        _prog_cache[key] = build_program(fvt, mask_list)
    nc = _prog_cache[key]

    in_maps = _prep_inputs(dec_inp, attn_mask, mems, Wq, Wkv, Wo, ln1_g, ln1_b,
                           W1, b1, W2, b2, ln2_g, ln2_b, mask_list)
    kw = {}
    if _trace:
        kw = dict(trace=True, **(_trace_kwargs or {}))
    res = run_bass_kernel_spmd(nc, in_maps, list(range(NCORES)), **kw)
    out = np.zeros((T, B, D), np.float32)
    for core in range(NCORES):
        b, g = core // G, core % G
        rows = np.concatenate(
            [np.arange(512 * q + 128 * g, 512 * q + 128 * g + 128)
             for q in range(4)])
        out[rows, b, :] = np.asarray(res.results[core]["out"]).reshape(TQ, D)
    if _trace:
        return out, res
    return out

